# revision 1
# baseline (speedup 1.0000x reference)
"""Trainium2 Bass kernel for nn_BlockwiseHadamardInputWrapper.

Computes out = (blockwise-Hadamard-128 of x along last dim) @ W.T + b
for x [2, 4096, 4096] f32, W [4096, 4096] f32, b [4096] f32.

Strategy (8 NeuronCores, data-parallel over the 8192 token rows):
  * Host: flatten x to [8192, 4096], shard 1024 rows per core, and
    pre-transpose each shard to xT [4096, 1024] so the contraction dim
    lands on SBUF partitions. W is transposed to wT [4096(in), 4096(out)]
    and pre-scaled by 1/sqrt(128) so the device can use the exact
    (+-1-valued) unnormalized Sylvester Hadamard matrix.
  * Device phase A: xhadT[k-block] = Hn @ xT[k-block] on the PE
    (one 128x128 stationary Hn, 512-wide moving slices), evicted
    PSUM->SBUF with rounding to float32r. Exact arithmetic (+-1 weights,
    fp32 accumulate).
  * Device phase B: out[tok, outf] = sum_k xhadT[k].T @ wT[k] using
    8 resident PSUM banks (one per 128-token tile) and k-contiguous
    matmul streams; wT is streamed from HBM exactly once. Bias is added
    by the DVE during PSUM eviction.
All matmuls run in float32r (full bf16-rate on the PE, ~11-bit operand
mantissa, fp32 accumulation).
"""

import numpy as np

import concourse.mybir as mybir
import concourse.tile as tile
from concourse import bacc
from concourse.bass_utils import run_bass_kernel_spmd

N_CORES = 8
B, S, D, O = 2, 4096, 4096, 4096
TOK = B * S                # 8192 token rows
TOK_PC = TOK // N_CORES    # 1024 per core
BLOCK = 128
NK = D // BLOCK            # 32 contraction blocks
NM = TOK_PC // 128         # 8 token tiles per core
NCH = 512                  # out-feature chunk (one PSUM bank in f32)
NN = O // NCH              # 8 out-feature chunks

_F32 = mybir.dt.float32
_F32R = mybir.dt.float32r


def _hadamard_pm1(n: int) -> np.ndarray:
    """Unnormalized (+-1) Sylvester Hadamard matrix."""
    H = np.array([[1.0]], dtype=np.float32)
    while H.shape[0] < n:
        H = np.block([[H, H], [H, -H]])
    return H.astype(np.float32)


def build_nc():
    nc = bacc.Bacc("TRN2", target_bir_lowering=False, debug=False,
                   num_devices=N_CORES)
    xT = nc.dram_tensor("xT", [D, TOK_PC], _F32R, kind="ExternalInput")
    wT = nc.dram_tensor("wT", [D, O], _F32R, kind="ExternalInput")
    bias = nc.dram_tensor("bias", [128, O], _F32, kind="ExternalInput")
    hmat = nc.dram_tensor("hmat", [BLOCK, BLOCK], _F32R, kind="ExternalInput")
    out = nc.dram_tensor("out", [TOK_PC, O], _F32, kind="ExternalOutput")

    with tile.TileContext(nc) as tc:
        with tc.tile_pool(name="const", bufs=1) as const:
            h_sb = const.tile([BLOCK, BLOCK], _F32R)
            nc.sync.dma_start(out=h_sb[:], in_=hmat[:])
            bias_sb = const.tile([128, O], _F32)
            nc.sync.dma_start(out=bias_sb[:], in_=bias[:])

            with tc.tile_pool(name="xhad", bufs=1) as xhp:
                xhad = xhp.tile([128, NK, TOK_PC], _F32R)

                # Phase A: xhadT[k] = Hn @ xT[k]  (exact: +-1 weights)
                with tc.tile_pool(name="xtp", bufs=3) as xtp, \
                     tc.tile_pool(name="psA", bufs=4, space="PSUM") as psa:
                    for k in range(NK):
                        xt_k = xtp.tile([128, TOK_PC], _F32R, name=f"xt{k}",
                                        tag="xt")
                        nc.sync.dma_start(
                            out=xt_k[:], in_=xT[k * 128:(k + 1) * 128, :])
                        for c in range(TOK_PC // NCH):
                            ps = psa.tile([128, NCH], _F32,
                                          name=f"psA{k}_{c}", tag="psA")
                            nc.tensor.matmul(
                                ps[:], h_sb[:],
                                xt_k[:, c * NCH:(c + 1) * NCH],
                                start=True, stop=True)
                            nc.vector.tensor_copy(
                                xhad[:, k, c * NCH:(c + 1) * NCH], ps[:])

                # Phase B: out[m-tile, n-chunk] = sum_k xhad[k,m].T @ wT[k,n]
                with tc.tile_pool(name="wtp", bufs=4) as wtp, \
                     tc.tile_pool(name="psB", bufs=1, space="PSUM") as psb, \
                     tc.tile_pool(name="outp", bufs=3) as outp:
                    for n in range(NN):
                        pss = [psb.tile([128, NCH], _F32, name=f"psB{n}_{m}",
                                        tag=f"psB{m}") for m in range(NM)]
                        for k in range(NK):
                            wt_t = wtp.tile([128, NCH], _F32R,
                                            name=f"wt{n}_{k}", tag="wt")
                            nc.sync.dma_start(
                                out=wt_t[:],
                                in_=wT[k * 128:(k + 1) * 128,
                                       n * NCH:(n + 1) * NCH])
                            for m in range(NM):
                                nc.tensor.matmul(
                                    pss[m][:],
                                    xhad[:, k, m * 128:(m + 1) * 128],
                                    wt_t[:],
                                    start=(k == 0), stop=(k == NK - 1),
                                    skip_group_check=True)
                        for m in range(NM):
                            ot = outp.tile([128, NCH], _F32,
                                           name=f"ot{n}_{m}", tag="ot")
                            nc.vector.tensor_add(
                                ot[:], pss[m][:],
                                bias_sb[:, n * NCH:(n + 1) * NCH])
                            nc.sync.dma_start(
                                out=out[m * 128:(m + 1) * 128,
                                        n * NCH:(n + 1) * NCH],
                                in_=ot[:])
    nc.compile()
    return nc


_NC_CACHE = None


def _get_nc():
    global _NC_CACHE
    if _NC_CACHE is None:
        _NC_CACHE = build_nc()
    return _NC_CACHE


def make_in_maps(x: np.ndarray, W: np.ndarray, b: np.ndarray):
    xf = np.ascontiguousarray(x.reshape(TOK, D).astype(np.float32, copy=False))
    # Fold the Hadamard normalization (1/sqrt(BLOCK)) into W.
    wTs = np.ascontiguousarray(
        (W.astype(np.float32, copy=False).T * np.float32(1.0 / np.sqrt(BLOCK))))
    bias_rep = np.ascontiguousarray(
        np.broadcast_to(b.astype(np.float32, copy=False)[None, :], (128, O)))
    hmat = _hadamard_pm1(BLOCK)
    in_maps = []
    for c in range(N_CORES):
        xTc = np.ascontiguousarray(xf[c * TOK_PC:(c + 1) * TOK_PC, :].T)
        in_maps.append(
            {"xT": xTc, "wT": wTs, "bias": bias_rep, "hmat": hmat})
    return in_maps


def run(x, W, b, trace=False):
    nc = _get_nc()
    in_maps = make_in_maps(x, W, b)
    res = run_bass_kernel_spmd(nc, in_maps, list(range(N_CORES)), trace=trace)
    parts = [res.results[c]["out"] for c in range(N_CORES)]
    full = np.concatenate(parts, axis=0).reshape(B, S, O)
    return full, res


def kernel(x: np.ndarray, W: np.ndarray, b: np.ndarray) -> np.ndarray:
    out, _ = run(x, W, b, trace=False)
    return out


# revision 2
# speedup vs baseline: 1.0310x; 1.0310x over previous
"""Trainium2 Bass kernel for nn_BlockwiseHadamardInputWrapper.

Computes out = (blockwise-Hadamard-128 of x along last dim) @ W.T + b
for x [2, 4096, 4096] f32, W [4096, 4096] f32, b [4096] f32.

Strategy (8 NeuronCores, data-parallel over the 8192 token rows):
  * Host: flatten x to [8192, 4096], shard 1024 rows per core, and
    pre-transpose each shard to xT [4096, 1024] so the contraction dim
    lands on SBUF partitions. W is transposed to wT [4096(in), 4096(out)]
    and pre-scaled by 1/sqrt(128) so the device can use the exact
    (+-1-valued) unnormalized Sylvester Hadamard matrix.
  * Device phase A: xhadT[k-block] = Hn @ xT[k-block] on the PE
    (one 128x128 stationary Hn, 512-wide moving slices), evicted
    PSUM->SBUF with rounding to float32r. Exact arithmetic (+-1 weights,
    fp32 accumulate). Phase A is software-pipelined INTO the first
    out-feature chunk's accumulation pass (m-tiles 0-3 only, so
    2 transform banks + 4 accumulation banks fit in PSUM) to keep the
    PE dense from the start.
  * Device phase B: out[tok, outf] = sum_k xhadT[k].T @ wT[k] using
    resident PSUM banks (one per 128-token tile) and k-contiguous
    matmul streams; wT is streamed from HBM (once, plus an 8 MiB
    re-read of the n=0 slab for the second m-half). Bias is added by
    the DVE during PSUM eviction.
All matmuls run in float32r (full bf16-rate on the PE, ~11-bit operand
mantissa, fp32 accumulation).
"""

import numpy as np

import concourse.mybir as mybir
import concourse.tile as tile
from concourse import bacc
from concourse.bass_utils import run_bass_kernel_spmd

N_CORES = 8
B, S, D, O = 2, 4096, 4096, 4096
TOK = B * S                # 8192 token rows
TOK_PC = TOK // N_CORES    # 1024 per core
BLOCK = 128
NK = D // BLOCK            # 32 contraction blocks
NM = TOK_PC // 128         # 8 token tiles per core
NCH = 512                  # out-feature chunk (one PSUM bank in f32)
NN = O // NCH              # 8 out-feature chunks

_F32 = mybir.dt.float32
_F32R = mybir.dt.float32r


def _hadamard_pm1(n: int) -> np.ndarray:
    """Unnormalized (+-1) Sylvester Hadamard matrix."""
    H = np.array([[1.0]], dtype=np.float32)
    while H.shape[0] < n:
        H = np.block([[H, H], [H, -H]])
    return H.astype(np.float32)


def build_nc():
    nc = bacc.Bacc("TRN2", target_bir_lowering=False, debug=False,
                   num_devices=N_CORES)
    xT = nc.dram_tensor("xT", [D, TOK_PC], _F32R, kind="ExternalInput")
    wT = nc.dram_tensor("wT", [D, O], _F32R, kind="ExternalInput")
    bias = nc.dram_tensor("bias", [128, O], _F32, kind="ExternalInput")
    hmat = nc.dram_tensor("hmat", [BLOCK, BLOCK], _F32R, kind="ExternalInput")
    out = nc.dram_tensor("out", [TOK_PC, O], _F32, kind="ExternalOutput")

    with tile.TileContext(nc) as tc:
        with tc.tile_pool(name="const", bufs=1) as const:
            h_sb = const.tile([BLOCK, BLOCK], _F32R)
            nc.sync.dma_start(out=h_sb[:], in_=hmat[:])
            bias_sb = const.tile([128, O], _F32)
            nc.sync.dma_start(out=bias_sb[:], in_=bias[:])

            with tc.tile_pool(name="xhad", bufs=1) as xhp:
                xhad = xhp.tile([128, NK, TOK_PC], _F32R)

                def emit_mm(pss, n, k, m, wt_t):
                    nc.tensor.matmul(
                        pss[m][:],
                        xhad[:, k, m * 128:(m + 1) * 128],
                        wt_t[:],
                        start=(k == 0), stop=(k == NK - 1),
                        skip_group_check=True)

                def emit_evict(pss, n, m, outp):
                    ot = outp.tile([128, NCH], _F32, name=f"ot{n}_{m}",
                                   tag="ot")
                    nc.vector.tensor_add(
                        ot[:], pss[m][:], bias_sb[:, n * NCH:(n + 1) * NCH])
                    nc.sync.dma_start(
                        out=out[m * 128:(m + 1) * 128,
                                n * NCH:(n + 1) * NCH],
                        in_=ot[:])

                def emit_wt_dma(wtp, n, k, uid):
                    wt_t = wtp.tile([128, NCH], _F32R, name=f"wt{uid}",
                                    tag="wt")
                    nc.sync.dma_start(
                        out=wt_t[:],
                        in_=wT[k * 128:(k + 1) * 128,
                               n * NCH:(n + 1) * NCH])
                    return wt_t

                # ---- Fused pass: phase A + (n=0, m-tiles 0-3) ----
                # Software-pipelined: B-mms for k lag the A-transform of
                # k by one step so the PE never waits on a fresh evict.
                with tc.tile_pool(name="xtp", bufs=3) as xtp, \
                     tc.tile_pool(name="psA", bufs=2, space="PSUM") as psa, \
                     tc.tile_pool(name="psH", bufs=1, space="PSUM") as psh, \
                     tc.tile_pool(name="wtp0", bufs=4) as wtp0, \
                     tc.tile_pool(name="outp0", bufs=2) as outp0:
                    pssh = [psh.tile([128, NCH], _F32, name=f"psH{m}",
                                     tag=f"psH{m}") for m in range(4)]
                    wt_prev = None
                    for k in range(NK + 1):
                        if k < NK:
                            xt_k = xtp.tile([128, TOK_PC], _F32R,
                                            name=f"xt{k}", tag="xt")
                            nc.sync.dma_start(
                                out=xt_k[:],
                                in_=xT[k * 128:(k + 1) * 128, :])
                            for c in range(TOK_PC // NCH):
                                ps = psa.tile([128, NCH], _F32,
                                              name=f"psA{k}_{c}", tag="psA")
                                nc.tensor.matmul(
                                    ps[:], h_sb[:],
                                    xt_k[:, c * NCH:(c + 1) * NCH],
                                    start=True, stop=True)
                                nc.vector.tensor_copy(
                                    xhad[:, k, c * NCH:(c + 1) * NCH], ps[:])
                        if k >= 1:
                            wt_prev = emit_wt_dma(wtp0, 0, k - 1, f"0a_{k-1}")
                            for m in range(4):
                                emit_mm(pssh, 0, k - 1, m, wt_prev)
                    for m in range(4):
                        emit_evict(pssh, 0, m, outp0)

                # ---- Remaining passes ----
                with tc.tile_pool(name="wtp", bufs=4) as wtp, \
                     tc.tile_pool(name="psB", bufs=1, space="PSUM") as psb, \
                     tc.tile_pool(name="outp", bufs=3) as outp:
                    # n=0, m-tiles 4-7 (re-streams the n=0 wT slab)
                    pss = [psb.tile([128, NCH], _F32, name=f"psB0_{m}",
                                    tag=f"psB{m}") for m in range(4, 8)]
                    pss = [None] * 4 + pss
                    for k in range(NK):
                        wt_t = emit_wt_dma(wtp, 0, k, f"0b_{k}")
                        for m in range(4, 8):
                            emit_mm(pss, 0, k, m, wt_t)
                    for m in range(4, 8):
                        emit_evict(pss, 0, m, outp)
                    # n = 1..7, full 8 m-tiles
                    for n in range(1, NN):
                        pss = [psb.tile([128, NCH], _F32, name=f"psB{n}_{m}",
                                        tag=f"psB{m}") for m in range(NM)]
                        for k in range(NK):
                            wt_t = emit_wt_dma(wtp, n, k, f"{n}_{k}")
                            for m in range(NM):
                                emit_mm(pss, n, k, m, wt_t)
                        for m in range(NM):
                            emit_evict(pss, n, m, outp)
    nc.compile()
    return nc


_NC_CACHE = None


def _get_nc():
    global _NC_CACHE
    if _NC_CACHE is None:
        _NC_CACHE = build_nc()
    return _NC_CACHE


def make_in_maps(x: np.ndarray, W: np.ndarray, b: np.ndarray):
    xf = np.ascontiguousarray(x.reshape(TOK, D).astype(np.float32, copy=False))
    # Fold the Hadamard normalization (1/sqrt(BLOCK)) into W.
    wTs = np.ascontiguousarray(
        (W.astype(np.float32, copy=False).T * np.float32(1.0 / np.sqrt(BLOCK))))
    bias_rep = np.ascontiguousarray(
        np.broadcast_to(b.astype(np.float32, copy=False)[None, :], (128, O)))
    hmat = _hadamard_pm1(BLOCK)
    in_maps = []
    for c in range(N_CORES):
        xTc = np.ascontiguousarray(xf[c * TOK_PC:(c + 1) * TOK_PC, :].T)
        in_maps.append(
            {"xT": xTc, "wT": wTs, "bias": bias_rep, "hmat": hmat})
    return in_maps


def run(x, W, b, trace=False):
    nc = _get_nc()
    in_maps = make_in_maps(x, W, b)
    res = run_bass_kernel_spmd(nc, in_maps, list(range(N_CORES)), trace=trace)
    parts = [res.results[c]["out"] for c in range(N_CORES)]
    full = np.concatenate(parts, axis=0).reshape(B, S, O)
    return full, res


def kernel(x: np.ndarray, W: np.ndarray, b: np.ndarray) -> np.ndarray:
    out, _ = run(x, W, b, trace=False)
    return out


# revision 5
# speedup vs baseline: 1.0992x; 1.0662x over previous
"""Trainium2 Bass kernel for nn_BlockwiseHadamardInputWrapper.

Computes out = (blockwise-Hadamard-128 of x along last dim) @ W.T + b
for x [2, 4096, 4096] f32, W [4096, 4096] f32, b [4096] f32.

Strategy (8 NeuronCores, data-parallel over the 8192 token rows):
  * Host: flatten x to [8192, 4096], shard 1024 rows per core, and
    pre-transpose each shard to xT [4096, 1024] so the contraction dim
    lands on SBUF partitions. W is transposed, pre-scaled by
    1/sqrt(128) (so the device can use the exact +-1-valued Sylvester
    Hadamard matrix), and stored in a [NK, NN, 128, 512]-tiled layout
    so every streamed weight tile is one fully contiguous 256 KiB read.
  * Device: a short PE warmup burst flips the HAM clock gate to
    2.4 GHz. Phase A (xhadT[k] = Hn @ xT[k], exact +-1 arithmetic,
    fp32 accumulate, rounded to float32r on eviction) is
    software-pipelined into the first out-feature chunk's accumulation
    pass (m-tiles 0-3 only, so transform banks + accumulation banks
    fit in the 8 PSUM banks). The remaining passes stream wT once
    (k-contiguous, 8 resident PSUM accumulators) at full PE rate.
    DMA dispatch is spread across engines (weights: sync, x: scalar,
    outputs/bias: gpsimd) to avoid head-of-line blocking. Bias is
    replicated across partitions by GpSimd and added by the DVE during
    PSUM eviction.
All matmuls run in float32r (full bf16-rate on the PE, ~11-bit operand
mantissa, fp32 accumulation).
"""

import numpy as np

import concourse.mybir as mybir
import concourse.tile as tile
from concourse import bacc
from concourse.bass_utils import run_bass_kernel_spmd

N_CORES = 8
B, S, D, O = 2, 4096, 4096, 4096
TOK = B * S                # 8192 token rows
TOK_PC = TOK // N_CORES    # 1024 per core
BLOCK = 128
NK = D // BLOCK            # 32 contraction blocks
NM = TOK_PC // 128         # 8 token tiles per core
NCH = 512                  # out-feature chunk (one PSUM bank in f32)
NN = O // NCH              # 8 out-feature chunks
N_WARMUP = 40           # PE warmup matmuls (~4us) to flip the HAM gate

_F32 = mybir.dt.float32
_F32R = mybir.dt.float32r


def _hadamard_pm1(n: int) -> np.ndarray:
    """Unnormalized (+-1) Sylvester Hadamard matrix."""
    H = np.array([[1.0]], dtype=np.float32)
    while H.shape[0] < n:
        H = np.block([[H, H], [H, -H]])
    return H.astype(np.float32)


def build_nc():
    nc = bacc.Bacc("TRN2", target_bir_lowering=False, debug=False,
                   num_devices=N_CORES)
    xT = nc.dram_tensor("xT", [D, TOK_PC], _F32R, kind="ExternalInput")
    # W, transposed+scaled, tiled: [NK, NN, 128, NCH]
    wTt = nc.dram_tensor("wTt", [NK, NN, 128, NCH], _F32R,
                         kind="ExternalInput")
    bias = nc.dram_tensor("bias", [1, O], _F32, kind="ExternalInput")
    hmat = nc.dram_tensor("hmat", [BLOCK, BLOCK], _F32R, kind="ExternalInput")
    out = nc.dram_tensor("out", [TOK_PC, O], _F32, kind="ExternalOutput")

    with tile.TileContext(nc) as tc:
        with tc.tile_pool(name="const", bufs=1) as const:
            h_sb = const.tile([BLOCK, BLOCK], _F32R)
            nc.gpsimd.dma_start(out=h_sb[:], in_=hmat[:])
            bias1 = const.tile([1, O], _F32)
            nc.gpsimd.dma_start(out=bias1[:], in_=bias[:])
            bias_sb = const.tile([128, O], _F32)
            nc.gpsimd.partition_broadcast(bias_sb[:], bias1[:])

            with tc.tile_pool(name="xhad", bufs=1) as xhp:
                xhad = xhp.tile([128, NK, TOK_PC], _F32R)

                def emit_mm(pss, n, k, m, wt_t):
                    nc.tensor.matmul(
                        pss[m][:],
                        xhad[:, k, m * 128:(m + 1) * 128],
                        wt_t[:],
                        start=(k == 0), stop=(k == NK - 1),
                        skip_group_check=True)

                def emit_evict(pss, n, m, outp):
                    ot = outp.tile([128, NCH], _F32, name=f"ot{n}_{m}",
                                   tag="ot")
                    nc.vector.tensor_add(
                        ot[:], pss[m][:], bias_sb[:, n * NCH:(n + 1) * NCH])
                    nc.gpsimd.dma_start(
                        out=out[m * 128:(m + 1) * 128,
                                n * NCH:(n + 1) * NCH],
                        in_=ot[:])

                def emit_wt_dma(wtp, n, k, uid):
                    wt_t = wtp.tile([128, NCH], _F32R, name=f"wt{uid}",
                                    tag="wt")
                    nc.sync.dma_start(out=wt_t[:], in_=wTt[k, n])
                    return wt_t

                # ---- Fused pass: PE warmup + phase A + (n=0, m 0-3) ----
                with tc.tile_pool(name="xtp", bufs=4) as xtp, \
                     tc.tile_pool(name="psA", bufs=3, space="PSUM") as psa, \
                     tc.tile_pool(name="psH", bufs=1, space="PSUM") as psh, \
                     tc.tile_pool(name="psW", bufs=1, space="PSUM") as psw, \
                     tc.tile_pool(name="wtp0", bufs=8) as wtp0, \
                     tc.tile_pool(name="outp0", bufs=2) as outp0:
                    # Warmup: harmless matmuls on the (tiny) Hadamard tile
                    # to get ~5us of continuous PE activity ASAP so the HAM
                    # clock gate opens before the real work arrives.
                    wps = psw.tile([128, BLOCK], _F32)
                    for _ in range(N_WARMUP):
                        nc.tensor.matmul(
                            wps[:], h_sb[:], h_sb[:],
                            start=True, stop=True, skip_group_check=True)

                    pssh = [psh.tile([128, NCH], _F32, name=f"psH{m}",
                                     tag=f"psH{m}") for m in range(4)]
                    for k in range(NK + 1):
                        if k < NK:
                            xt_k = xtp.tile([128, TOK_PC], _F32R,
                                            name=f"xt{k}", tag="xt")
                            nc.scalar.dma_start(
                                out=xt_k[:],
                                in_=xT[k * 128:(k + 1) * 128, :])
                            for c in range(TOK_PC // NCH):
                                ps = psa.tile([128, NCH], _F32,
                                              name=f"psA{k}_{c}", tag="psA")
                                nc.tensor.matmul(
                                    ps[:], h_sb[:],
                                    xt_k[:, c * NCH:(c + 1) * NCH],
                                    start=True, stop=True)
                                nc.vector.tensor_copy(
                                    xhad[:, k, c * NCH:(c + 1) * NCH], ps[:])
                        if k >= 1:
                            wt_t = emit_wt_dma(wtp0, 0, k - 1, f"0a_{k-1}")
                            for m in range(4):
                                emit_mm(pssh, 0, k - 1, m, wt_t)
                    for m in range(4):
                        emit_evict(pssh, 0, m, outp0)

                # ---- Remaining passes ----
                with tc.tile_pool(name="wtp", bufs=8) as wtp, \
                     tc.tile_pool(name="psB", bufs=1, space="PSUM") as psb, \
                     tc.tile_pool(name="outp", bufs=3) as outp:
                    # n=0, m-tiles 4-7 (re-streams the n=0 wT slab)
                    pss = [psb.tile([128, NCH], _F32, name=f"psB0_{m}",
                                    tag=f"psB{m}") for m in range(4, 8)]
                    pss = [None] * 4 + pss
                    for k in range(NK):
                        wt_t = emit_wt_dma(wtp, 0, k, f"0b_{k}")
                        for m in range(4, 8):
                            emit_mm(pss, 0, k, m, wt_t)
                    for m in range(4, 8):
                        emit_evict(pss, 0, m, outp)
                    # n = 1..7, full 8 m-tiles
                    for n in range(1, NN):
                        pss = [psb.tile([128, NCH], _F32, name=f"psB{n}_{m}",
                                        tag=f"psB{m}") for m in range(NM)]
                        for k in range(NK):
                            wt_t = emit_wt_dma(wtp, n, k, f"{n}_{k}")
                            for m in range(NM):
                                emit_mm(pss, n, k, m, wt_t)
                        for m in range(NM):
                            emit_evict(pss, n, m, outp)
    nc.compile()
    return nc


_NC_CACHE = None


def _get_nc():
    global _NC_CACHE
    if _NC_CACHE is None:
        _NC_CACHE = build_nc()
    return _NC_CACHE


def make_in_maps(x: np.ndarray, W: np.ndarray, b: np.ndarray):
    xf = np.ascontiguousarray(x.reshape(TOK, D).astype(np.float32, copy=False))
    # Fold the Hadamard normalization (1/sqrt(BLOCK)) into W; transpose to
    # [in, out] and tile to [NK, NN, 128, NCH] for contiguous streaming.
    wTs = (W.astype(np.float32, copy=False).T
           * np.float32(1.0 / np.sqrt(BLOCK)))
    wTt = np.ascontiguousarray(
        wTs.reshape(NK, 128, NN, NCH).transpose(0, 2, 1, 3))
    bias1 = np.ascontiguousarray(
        b.astype(np.float32, copy=False).reshape(1, O))
    hmat = _hadamard_pm1(BLOCK)
    in_maps = []
    for c in range(N_CORES):
        xTc = np.ascontiguousarray(xf[c * TOK_PC:(c + 1) * TOK_PC, :].T)
        in_maps.append(
            {"xT": xTc, "wTt": wTt, "bias": bias1, "hmat": hmat})
    return in_maps


def run(x, W, b, trace=False):
    nc = _get_nc()
    in_maps = make_in_maps(x, W, b)
    res = run_bass_kernel_spmd(nc, in_maps, list(range(N_CORES)), trace=trace)
    parts = [res.results[c]["out"] for c in range(N_CORES)]
    full = np.concatenate(parts, axis=0).reshape(B, S, O)
    return full, res


def kernel(x: np.ndarray, W: np.ndarray, b: np.ndarray) -> np.ndarray:
    out, _ = run(x, W, b, trace=False)
    return out


# revision 6
# speedup vs baseline: 1.1091x; 1.0090x over previous
"""Trainium2 Bass kernel for nn_BlockwiseHadamardInputWrapper.

Computes out = (blockwise-Hadamard-128 of x along last dim) @ W.T + b
for x [2, 4096, 4096] f32, W [4096, 4096] f32, b [4096] f32.

Strategy (8 NeuronCores, data-parallel over the 8192 token rows):
  * Host: flatten x to [8192, 4096], shard 1024 rows per core, and
    pre-transpose each shard to xT [4096, 1024] so the contraction dim
    lands on SBUF partitions. W is transposed, pre-scaled by
    1/sqrt(128) (so the device can use the exact +-1-valued Sylvester
    Hadamard matrix), and stored in a [NK, NN, 128, 512]-tiled layout
    so every streamed weight tile is one fully contiguous 256 KiB read.
  * Device: a PE warmup burst (~7.5us of tiny matmuls) flips the HAM
    clock gate to 2.4 GHz while x streams in. Phase A computes
    xhadT[k] = Hn @ xT[k] (exact +-1 arithmetic, fp32 accumulate,
    rounded to float32r on eviction) as x arrives — the whole phase
    hides inside the unavoidable 16 MiB x inflow. Phase B then runs
    out[tok, outf] = sum_k xhadT[k].T @ wT[k] with 8 resident PSUM
    accumulators (one per 128-token tile), k-contiguous matmul streams,
    and wT streamed from HBM exactly once. Bias is replicated across
    partitions by GpSimd once and added by the DVE during PSUM
    eviction. DMA dispatch is spread across rings (weights+H: sync,
    x: scalar, bias/outputs: gpsimd+scalar) to avoid head-of-line
    blocking.
All matmuls run in float32r (full bf16-rate on the PE, ~11-bit operand
mantissa, fp32 accumulation).
"""

import numpy as np

import concourse.mybir as mybir
import concourse.tile as tile
from concourse import bacc
from concourse.bass_utils import run_bass_kernel_spmd

N_CORES = 8
B, S, D, O = 2, 4096, 4096, 4096
TOK = B * S                # 8192 token rows
TOK_PC = TOK // N_CORES    # 1024 per core
BLOCK = 128
NK = D // BLOCK            # 32 contraction blocks
NM = TOK_PC // 128         # 8 token tiles per core
NCH = 512                  # out-feature chunk (one PSUM bank in f32)
NN = O // NCH              # 8 out-feature chunks
N_WARMUP = 72              # PE warmup matmuls (~7.5us cold) for the HAM gate

_F32 = mybir.dt.float32
_F32R = mybir.dt.float32r


def _hadamard_pm1(n: int) -> np.ndarray:
    """Unnormalized (+-1) Sylvester Hadamard matrix."""
    H = np.array([[1.0]], dtype=np.float32)
    while H.shape[0] < n:
        H = np.block([[H, H], [H, -H]])
    return H.astype(np.float32)


def build_nc():
    nc = bacc.Bacc("TRN2", target_bir_lowering=False, debug=False,
                   num_devices=N_CORES)
    xT = nc.dram_tensor("xT", [D, TOK_PC], _F32R, kind="ExternalInput")
    # W, transposed+scaled, tiled: [NK, NN, 128, NCH]
    wTt = nc.dram_tensor("wTt", [NK, NN, 128, NCH], _F32R,
                         kind="ExternalInput")
    bias = nc.dram_tensor("bias", [1, O], _F32, kind="ExternalInput")
    hmat = nc.dram_tensor("hmat", [BLOCK, BLOCK], _F32R, kind="ExternalInput")
    out = nc.dram_tensor("out", [TOK_PC, O], _F32, kind="ExternalOutput")

    with tile.TileContext(nc) as tc:
        with tc.tile_pool(name="const", bufs=1) as const:
            # H first on the sync ring: the warmup matmuls need it ASAP.
            h_sb = const.tile([BLOCK, BLOCK], _F32R)
            nc.sync.dma_start(out=h_sb[:], in_=hmat[:])
            bias1 = const.tile([1, O], _F32)
            nc.gpsimd.dma_start(out=bias1[:], in_=bias[:])
            bias_sb = const.tile([128, O], _F32)
            nc.gpsimd.partition_broadcast(bias_sb[:], bias1[:])

            with tc.tile_pool(name="xhad", bufs=1) as xhp:
                xhad = xhp.tile([128, NK, TOK_PC], _F32R)

                # ---- Warmup + phase A (hidden under the x inflow) ----
                with tc.tile_pool(name="xtp", bufs=5) as xtp, \
                     tc.tile_pool(name="psA", bufs=3, space="PSUM") as psa, \
                     tc.tile_pool(name="psW", bufs=1, space="PSUM") as psw:
                    wps = psw.tile([128, BLOCK], _F32)
                    for _ in range(N_WARMUP):
                        nc.tensor.matmul(
                            wps[:], h_sb[:], h_sb[:],
                            start=True, stop=True, skip_group_check=True)
                    for k in range(NK):
                        xt_k = xtp.tile([128, TOK_PC], _F32R,
                                        name=f"xt{k}", tag="xt")
                        nc.scalar.dma_start(
                            out=xt_k[:], in_=xT[k * 128:(k + 1) * 128, :])
                        for c in range(TOK_PC // NCH):
                            ps = psa.tile([128, NCH], _F32,
                                          name=f"psA{k}_{c}", tag="psA")
                            nc.tensor.matmul(
                                ps[:], h_sb[:],
                                xt_k[:, c * NCH:(c + 1) * NCH],
                                start=True, stop=True)
                            nc.vector.tensor_copy(
                                xhad[:, k, c * NCH:(c + 1) * NCH], ps[:])

                # ---- Phase B: 8 passes over out-feature chunks ----
                with tc.tile_pool(name="wtp", bufs=12) as wtp, \
                     tc.tile_pool(name="psB", bufs=1, space="PSUM") as psb, \
                     tc.tile_pool(name="outp", bufs=4) as outp:
                    for n in range(NN):
                        pss = [psb.tile([128, NCH], _F32, name=f"psB{n}_{m}",
                                        tag=f"psB{m}") for m in range(NM)]
                        for k in range(NK):
                            wt_t = wtp.tile([128, NCH], _F32R,
                                            name=f"wt{n}_{k}", tag="wt")
                            nc.sync.dma_start(out=wt_t[:], in_=wTt[k, n])
                            for m in range(NM):
                                nc.tensor.matmul(
                                    pss[m][:],
                                    xhad[:, k, m * 128:(m + 1) * 128],
                                    wt_t[:],
                                    start=(k == 0), stop=(k == NK - 1),
                                    skip_group_check=True)
                        for m in range(NM):
                            ot = outp.tile([128, NCH], _F32,
                                           name=f"ot{n}_{m}", tag="ot")
                            nc.vector.tensor_add(
                                ot[:], pss[m][:],
                                bias_sb[:, n * NCH:(n + 1) * NCH])
                            eng = nc.gpsimd if m % 2 == 0 else nc.scalar
                            eng.dma_start(
                                out=out[m * 128:(m + 1) * 128,
                                        n * NCH:(n + 1) * NCH],
                                in_=ot[:])
    nc.compile()
    return nc


_NC_CACHE = None


def _get_nc():
    global _NC_CACHE
    if _NC_CACHE is None:
        _NC_CACHE = build_nc()
    return _NC_CACHE


def make_in_maps(x: np.ndarray, W: np.ndarray, b: np.ndarray):
    xf = np.ascontiguousarray(x.reshape(TOK, D).astype(np.float32, copy=False))
    # Fold the Hadamard normalization (1/sqrt(BLOCK)) into W; transpose to
    # [in, out] and tile to [NK, NN, 128, NCH] for contiguous streaming.
    wTs = (W.astype(np.float32, copy=False).T
           * np.float32(1.0 / np.sqrt(BLOCK)))
    wTt = np.ascontiguousarray(
        wTs.reshape(NK, 128, NN, NCH).transpose(0, 2, 1, 3))
    bias1 = np.ascontiguousarray(
        b.astype(np.float32, copy=False).reshape(1, O))
    hmat = _hadamard_pm1(BLOCK)
    in_maps = []
    for c in range(N_CORES):
        xTc = np.ascontiguousarray(xf[c * TOK_PC:(c + 1) * TOK_PC, :].T)
        in_maps.append(
            {"xT": xTc, "wTt": wTt, "bias": bias1, "hmat": hmat})
    return in_maps


def run(x, W, b, trace=False):
    nc = _get_nc()
    in_maps = make_in_maps(x, W, b)
    res = run_bass_kernel_spmd(nc, in_maps, list(range(N_CORES)), trace=trace)
    parts = [res.results[c]["out"] for c in range(N_CORES)]
    full = np.concatenate(parts, axis=0).reshape(B, S, O)
    return full, res


def kernel(x: np.ndarray, W: np.ndarray, b: np.ndarray) -> np.ndarray:
    out, _ = run(x, W, b, trace=False)
    return out
